# revision 32
# baseline (speedup 1.0000x reference)
"""BiLSTM-CRF Trainium2 kernel — time-sliced across 8 cores.

Each core owns a 64-timestep slice of the sequence and runs BOTH lstm
directions over the full batch (64), warming up from an exactly-forced
zero state W steps before its slice (LSTM state memory decays ~2x/step;
W=24 gives h error ~1e-6, far under the 2e-4 abs budget). This cuts the
sequential scan from 512 steps to 88 per core and widens every
instruction from 8 to 64 batch columns, amortizing the large fixed
per-instruction costs (ACT ~293ns, DVE ~150ns, PE ldweights ~104ns).

Contract: kernel(**inputs) takes FULL unsharded inputs, returns FULL
[B, T, TAGS, TAGS] crf_scores. Host only does weight re-layout, index
building, and output concatenation.
"""
import sys
import types
from contextlib import ExitStack

import ml_dtypes
import numpy as np

import concourse.bacc as bacc
import concourse.bass as bass
import concourse.mybir as mybir
import concourse.tile as tile
from concourse import library_config
from concourse.bass_utils import run_bass_kernel_spmd

# ---- problem dims (hardcoded per spec) ----
VOCAB = 30000
EMB = 256
HD = 128          # per-direction hidden
G4 = 512          # 4*HD gates
TAGS = 16
B, T = 64, 512
NCORES = 8

# ---- time-slice geometry ----
BC = B            # full batch on every core
OWN = 64          # owned timesteps per core
W = 16            # warmup steps (zero-state spin-up; h err ~3e-5 << budget)
SEQ = W + OWN     # scan steps per direction = 88
UB = SEQ // 8 + W // 8  # union blocks of 8 timesteps = 14
NBLK = SEQ // 8   # projection blocks per direction = 11
BOFF = W // 8     # b-window offset into union blocks = 3
NT = UB * 8 * BC  # tokens per core = 7168

BF16 = mybir.dt.bfloat16
F32 = mybir.dt.float32
I16 = mybir.dt.int16
AF = mybir.ActivationFunctionType
ALU = mybir.AluOpType

# gate order in reference (jnp.split): i, f, g, o. Reorder to [f, i, o, g]
# so one sigmoid covers [f|i|o] and g rides along as sigma(2x)
# (tanh(x) = 2*sigma(2x) - 1, g-weights doubled on host).
_PERM = np.concatenate([
    np.arange(128, 256),   # f
    np.arange(0, 128),     # i
    np.arange(384, 512),   # o
    np.arange(256, 384),   # g
])


def _ensure_ntff_hook():
    """The RL image's antenv lacks axon_hooks; inject it so trace=True works."""
    if "antenv.axon_hooks" in sys.modules:
        return
    mod = types.ModuleType("antenv.axon_hooks")
    mod._hook = None
    mod.set_axon_ntff_profile_hook = lambda h: setattr(mod, "_hook", h)
    mod.get_axon_ntff_profile_hook = lambda: mod._hook
    sys.modules["antenv.axon_hooks"] = mod
    try:
        import antenv
        antenv.axon_hooks = mod
        from trn_agent_boot.trn_boot import _ntff_profile_via_ctypes
        mod.set_axon_ntff_profile_hook(
            _ntff_profile_via_ctypes("/opt/axon/libaxon_pjrt.so"))
    except Exception:
        pass


# h-history piece boundaries per direction (tiles split so the emission
# epilogue's tile-granular deps bind near each chunk's true readiness).
# f writes cols ascending (col j+1 at step j), emission reads cols 25..88.
# b writes cols descending (col 87-j at step j), emission reads cols 0..63.
F_PIECES = [0, W + 33, W + 47, W + 57, SEQ + 1]
B_PIECES = [0, 10, 20, 34, SEQ + 1]


def build(seq: int = SEQ, bc: int = BC):
    nc = bacc.Bacc("TRN2", target_bir_lowering=False, debug=False)

    # ---- DRAM I/O ----
    emb_d = nc.dram_tensor("emb", [VOCAB, EMB], BF16, kind="ExternalInput")
    idx_d = nc.dram_tensor("idx", [128, NT // 16], I16, kind="ExternalInput")
    wihT_d = {d: nc.dram_tensor(f"wihT_{d}", [EMB, G4], BF16, kind="ExternalInput")
              for d in "fb"}
    whhT_d = {d: nc.dram_tensor(f"whhT_{d}", [HD, G4], BF16, kind="ExternalInput")
              for d in "fb"}
    bias_d = {d: nc.dram_tensor(f"bias_{d}", [128, 4], F32, kind="ExternalInput")
              for d in "fb"}
    biasw_d = {d: nc.dram_tensor(f"biasw_{d}", [128, 4], F32, kind="ExternalInput")
               for d in "fb"}
    woutT_d = nc.dram_tensor("woutT", [2, HD, TAGS], BF16, kind="ExternalInput")
    trans_d = nc.dram_tensor("trans", [128, TAGS * TAGS], F32, kind="ExternalInput")
    ident_d = nc.dram_tensor("ident", [128, 128], BF16, kind="ExternalInput")
    crf_d = nc.dram_tensor("crf", [OWN * bc, TAGS * TAGS], F32, kind="ExternalOutput")

    with tile.TileContext(nc) as tc, ExitStack() as ctx:
        const = ctx.enter_context(tc.tile_pool(name="const", bufs=1))
        big = ctx.enter_context(tc.tile_pool(name="big", bufs=1))

        # ---- persistent SBUF ----
        idx_sb = const.tile([128, NT // 16], I16)
        wihT = {d: const.tile([128, 2, G4], BF16, tag=f"wihT{d}", name=f"wihT{d}") for d in "fb"}
        whhT = {d: const.tile([HD, G4], BF16, tag=f"whhT{d}", name=f"whhT{d}") for d in "fb"}
        bias = {d: const.tile([128, 4], F32, tag=f"bias{d}", name=f"bias{d}") for d in "fb"}
        biasw = {d: const.tile([128, 4], F32, tag=f"biasw{d}", name=f"biasw{d}") for d in "fb"}
        woutT = const.tile([HD, 2, TAGS], BF16)
        trans = const.tile([128, TAGS * TAGS], F32)
        ident = const.tile([128, 128], BF16)
        C2 = {d: const.tile([128, 2 * bc], F32, tag=f"C2{d}", name=f"C2{d}")
              for d in "fb"}

        xT = big.tile([128, UB, 2, 512], BF16, tag="xT")
        zin = {d: big.tile([128, seq, 4 * bc], BF16, tag=f"zin{d}", name=f"zin{d}")
               for d in "fb"}

        # h histories as piece-split tiles
        hp = {}
        for d, bounds in (("f", F_PIECES), ("b", B_PIECES)):
            hp[d] = [(bounds[i], bounds[i + 1],
                      big.tile([128, bounds[i + 1] - bounds[i], bc], BF16,
                               tag=f"h{d}{i}", name=f"h{d}{i}"))
                     for i in range(len(bounds) - 1)]

        def hcol(d, col):
            for lo, hi, t in hp[d]:
                if lo <= col < hi:
                    return t[:, col - lo, :]
            raise AssertionError((d, col))

        def hspan(d, c0, n):
            for lo, hi, t in hp[d]:
                if lo <= c0 and c0 + n <= hi:
                    return t[:, c0 - lo:c0 - lo + n, :]
            raise AssertionError((d, c0, n))

        # ---- load constants ----
        nc.sync.dma_start(idx_sb[:], idx_d[:])
        for d in "fb":
            nc.sync.dma_start(wihT[d][:], wihT_d[d].rearrange("(k p) g -> p k g", p=128))
            nc.sync.dma_start(whhT[d][:], whhT_d[d][:])
            nc.sync.dma_start(bias[d][:], bias_d[d][:])
            nc.sync.dma_start(biasw[d][:], biasw_d[d][:])
        nc.sync.dma_start(woutT[:], woutT_d.rearrange("c h t -> h c t"))
        nc.sync.dma_start(trans[:], trans_d[:])
        nc.sync.dma_start(ident[:], ident_d[:])
        for d in "fb":
            nc.vector.memset(C2[d][:], 0.0)
        nc.gpsimd.memset(hcol("f", 0), 0.0)
        nc.gpsimd.memset(hcol("b", seq), 0.0)

        # ---- embedding gathers, in two-ended consumption order ----
        # f consumes union blocks 0,1,2,... ; b consumes 13,12,...
        gorder = []
        for k in range((UB + 1) // 2):
            gorder.append(k)
            if UB - 1 - k != k:
                gorder.append(UB - 1 - k)
        nc.gpsimd.load_library(library_config.mlp)
        for u in gorder:
            nc.gpsimd.dma_gather(
                xT[:, u, :, :], emb_d[:, :], idx_sb[:, 32 * u:32 * (u + 1)],
                512, 512, EMB, transpose=True)

        # ---- pools for projection + scan + emission (all open together
        # so no pool-release serialization mid-stream) ----
        # PSUM is 8 banks; pool bufs are bank-granular. matmul start=True
        # clears the WHOLE bank, so each direction's z needs its own bank.
        zpsum = ctx.enter_context(tc.tile_pool(name="zpsum", bufs=2, space="PSUM"))
        spsum = ctx.enter_context(tc.tile_pool(name="spsum", bufs=4, space="PSUM"))
        epsum = ctx.enter_context(tc.tile_pool(name="epsum", bufs=2, space="PSUM"))
        sZ = ctx.enter_context(tc.tile_pool(name="sZ", bufs=4))
        sS = ctx.enter_context(tc.tile_pool(name="sS", bufs=4))
        sP = ctx.enter_context(tc.tile_pool(name="sP", bufs=4))
        sT = ctx.enter_context(tc.tile_pool(name="sT", bufs=4))
        ecrf = ctx.enter_context(tc.tile_pool(name="ecrf", bufs=4))

        def proj_chunk(d, p, c):
            """Input projection for direction d, block p (8 timesteps),
            gate chunk c: 2 matmuls + 1 biased copyback. f reads union
            block p; b reads union block p+BOFF. Warmup blocks use the
            warm bias variant (edge cores force-kill f/i gates -> exact
            zero state)."""
            u = p if d == "f" else p + BOFF
            warm = (p < W // 8) if d == "f" else (p >= NBLK - W // 8)
            bsel = biasw[d] if warm else bias[d]
            zp = zpsum.tile([128, 512], F32, tag="zp")
            nc.tensor.matmul(zp[:], wihT[d][:, 0, 128 * c:128 * (c + 1)],
                             xT[:, u, 0, :], start=True, stop=False)
            nc.tensor.matmul(zp[:], wihT[d][:, 1, 128 * c:128 * (c + 1)],
                             xT[:, u, 1, :], start=False, stop=True)
            nc.scalar.activation(
                zin[d][:, 8 * p:8 * (p + 1), bc * c:bc * (c + 1)], zp[:],
                AF.Identity, bias=bsel[:, c:c + 1])

        # remaining projection work, spread chunk-by-chunk through the scan
        # (keeps the PE stream dense so the HAM clock stays warm); block
        # order follows two-ended consumption.
        proj_tasks = []
        fseq = list(range(2, NBLK))
        bseq = list(range(NBLK - 3, -1, -1))
        for i in range(NBLK - 2):
            proj_tasks += [("f", fseq[i], c) for c in range(4)]
            proj_tasks += [("b", bseq[i], c) for c in range(4)]
        proj_tasks.reverse()  # pop() from the front of the schedule

        # prime the pipeline: first two blocks per direction
        for p in (0, 1):
            for c in range(4):
                proj_chunk("f", p, c)
        for p in (NBLK - 1, NBLK - 2):
            for c in range(4):
                proj_chunk("b", p, c)

        # ---- the recurrent scan (fwd + bwd interleaved) ----
        def new_z(j):
            """Fresh psum tiles for step j. The f-dir gets its zin injected
            on the PE (identity mm, pre-issued); the b-dir's zin is added on
            the DVE instead (halves the PE injection load; one direction
            keeps the short PE-only critical path)."""
            zt = {}
            for d, col in (("f", j), ("b", seq - 1 - j)):
                zt[d] = spsum.tile([128, 4 * bc], F32, tag="z", name=f"z{d}")
                if d == "f":
                    nc.tensor.matmul(zt[d][:], ident[:],
                                     zin[d][:, col, :], start=True, stop=False)
            return zt

        z = new_z(0)
        for j in range(seq):
            u = seq - 1 - j
            # feed projection chunks into the instruction stream
            if proj_tasks:
                proj_chunk(*proj_tasks.pop())
            else:
                # HAM filler: a large-N matmul with no consumer keeps the
                # PE array streaming so the clock gate stays at 8/8
                # (small-N scan matmuls alone read as idle and the PE
                # drops to 1.2 GHz).
                fl = zpsum.tile([128, 512], F32, tag="zp")
                nc.tensor.matmul(fl[:], ident[:], xT[:, 0, 0, :],
                                 start=True, stop=True)
            for d, rd_col in (("f", j), ("b", u + 1)):
                for c in range(4):
                    nc.tensor.matmul(
                        z[d][:, bc * c:bc * (c + 1)],
                        whhT[d][:, 128 * c:128 * (c + 1)],
                        hcol(d, rd_col),
                        start=(d == "b" and c == 0), stop=(c == 3))
            z_cur, z = z, (new_z(j + 1) if j + 1 < seq else None)
            for d, wr_col in (("f", j + 1), ("b", u)):
                if d == "b":
                    zs = sZ.tile([128, 4 * bc], F32, tag="zs")
                    nc.vector.tensor_tensor(zs[:], z_cur[d][:],
                                            zin[d][:, u, :], ALU.add)
                    zsrc = zs
                else:
                    zsrc = z_cur[d]
                S = sS.tile([128, 4 * bc], F32, tag="S")
                nc.scalar.activation(S[:], zsrc[:], AF.Sigmoid)
                # gtilde = 2*sigma(2 z_g) - 1
                nc.vector.tensor_scalar(
                    C2[d][:, bc:2 * bc], S[:, 3 * bc:4 * bc], 2.0, -1.0,
                    ALU.mult, ALU.add)
                # [sf*c | si*gtilde]
                P2 = sP.tile([128, 2 * bc], F32, tag="P2")
                nc.vector.tensor_tensor(P2[:], S[:, 0:2 * bc], C2[d][:],
                                        ALU.mult)
                nc.vector.tensor_tensor(C2[d][:, 0:bc], P2[:, 0:bc],
                                        P2[:, bc:2 * bc], ALU.add)
                TC = sT.tile([128, bc], F32, tag="TC")
                nc.scalar.activation(TC[:], C2[d][:, 0:bc], AF.Tanh)
                nc.vector.tensor_tensor(hcol(d, wr_col),
                                        S[:, 2 * bc:3 * bc], TC[:], ALU.mult)

        # ---- emission + CRF broadcast-add + store ----
        # chunk m covers owned timesteps {2m, 2m+1}; ready at scan step
        # max(25+2m, 87-2m) -> emit middle-out.
        order = sorted(range(OWN // 2),
                       key=lambda m: max(W + 1 + 2 * m, seq - 1 - 2 * m))
        for m in order:
            e = epsum.tile([128, TAGS], F32, tag="e")
            nc.tensor.matmul(e[:], hspan("f", W + 1 + 2 * m, 2),
                             woutT[:, 0, :], start=True, stop=False)
            nc.tensor.matmul(e[:], hspan("b", 2 * m, 2),
                             woutT[:, 1, :], start=False, stop=True)
            crf_sb = ecrf.tile([128, TAGS * TAGS], F32, tag="crf")
            e_b = e[:, None, :].to_broadcast([128, TAGS, TAGS])
            nc.vector.tensor_tensor(crf_sb[:], e_b, trans[:], ALU.add)
            nc.sync.dma_start(crf_d[128 * m:128 * (m + 1), :], crf_sb[:])

    nc.compile()
    _assert_ldw_pairing(nc)
    return nc


def _assert_ldw_pairing(nc):
    """Every non-self-loading matmul must directly follow an InstLdweights
    whose weights AP matches the matmul's weights operand."""
    for f in nc.m.functions:
        for bb in f.blocks:
            prev_pe = None
            for ins in bb.instructions:
                if ins.engine != mybir.EngineType.PE:
                    continue
                if isinstance(ins, mybir.InstMatmult) and ins.ldweights is False:
                    assert isinstance(prev_pe, mybir.InstLdweights), (
                        f"{ins.name}: non-self-loading matmul not preceded by "
                        f"ldweights (got {type(prev_pe).__name__})")
                    assert repr(prev_pe.ins[0]) == repr(ins.ins[1]), (
                        f"{ins.name}: weights mismatch with {prev_pe.name}")
                prev_pe = ins


_CACHE = {}


def _get_nc():
    if "nc" not in _CACHE:
        _CACHE["nc"] = build()
    return _CACHE["nc"]


def _prep_dir(w_ih, w_hh, b):
    w_ih = np.asarray(w_ih, np.float32)[_PERM].copy()
    w_hh = np.asarray(w_hh, np.float32)[_PERM].copy()
    b = np.asarray(b, np.float32)[_PERM].copy()
    w_ih[384:512] *= 2.0
    w_hh[384:512] *= 2.0
    b[384:512] *= 2.0
    wihT = np.ascontiguousarray(w_ih.T).astype(ml_dtypes.bfloat16)
    whhT = np.ascontiguousarray(w_hh.T).astype(ml_dtypes.bfloat16)
    bias = np.ascontiguousarray(b.reshape(4, 128).T).astype(np.float32)
    return wihT, whhT, bias


def make_in_maps(sentences, embedding, W_ih_f, W_hh_f, b_f, W_ih_b, W_hh_b,
                 b_b, W_out, b_out, transition):
    emb = np.asarray(embedding, np.float32).astype(ml_dtypes.bfloat16)
    wihT_f, whhT_f, bias_f = _prep_dir(W_ih_f, W_hh_f, b_f)
    wihT_b, whhT_b, bias_b = _prep_dir(W_ih_b, W_hh_b, b_b)
    wo = np.asarray(W_out, np.float32)  # [16, 256]
    woutT = np.stack([np.ascontiguousarray(wo[:, :128].T),
                      np.ascontiguousarray(wo[:, 128:].T)])
    woutT = woutT.astype(ml_dtypes.bfloat16)  # [2, 128, 16]
    trans_aug = (np.asarray(transition, np.float32)
                 + np.asarray(b_out, np.float32)[None, :]).reshape(-1)
    trans_rep = np.ascontiguousarray(
        np.broadcast_to(trans_aug, (128, 256))).astype(np.float32)
    ident = np.eye(128, dtype=ml_dtypes.bfloat16)

    sent = np.asarray(sentences).astype(np.int64)
    in_maps = []
    for s in range(NCORES):
        # union token window [64s - W, 64s + SEQ), clamped; (t, b) order
        ts = np.clip(np.arange(64 * s - W, 64 * s + SEQ), 0, T - 1)
        toks = sent[:, ts].T.reshape(-1)  # [NT] t-major
        idx = np.tile(toks.reshape(NT // 16, 16).T.astype(np.int16), (8, 1))
        # warm bias kills f/i gates only where the warmup window falls
        # outside the real sequence (slice edges)
        bwf = bias_f.copy()
        bwb = bias_b.copy()
        if s == 0:
            bwf[:, 0:2] = -30000.0
        if s == NCORES - 1:
            bwb[:, 0:2] = -30000.0
        in_maps.append({
            "emb": emb, "idx": idx,
            "wihT_f": wihT_f, "wihT_b": wihT_b,
            "whhT_f": whhT_f, "whhT_b": whhT_b,
            "bias_f": bias_f, "bias_b": bias_b,
            "biasw_f": bwf, "biasw_b": bwb,
            "woutT": woutT, "trans": trans_rep, "ident": ident,
        })
    return in_maps


def assemble_out(results):
    out = np.empty((B, T, TAGS, TAGS), np.float32)
    for s in range(NCORES):
        crf = results[s]["crf"].reshape(OWN, B, TAGS, TAGS)
        out[:, OWN * s:OWN * (s + 1)] = crf.transpose(1, 0, 2, 3)
    return out


def kernel(**inputs):
    _ensure_ntff_hook()
    nc = _get_nc()
    in_maps = make_in_maps(**inputs)
    res = run_bass_kernel_spmd(nc, in_maps, list(range(NCORES)))
    return assemble_out(res.results)


# revision 33
# speedup vs baseline: 1.1324x; 1.1324x over previous
"""BiLSTM-CRF Trainium2 kernel — time-sliced across 8 cores.

Each core owns a 64-timestep slice of the sequence and runs BOTH lstm
directions over the full batch (64), warming up from an exactly-forced
zero state W steps before its slice (LSTM state memory decays ~2x/step;
W=24 gives h error ~1e-6, far under the 2e-4 abs budget). This cuts the
sequential scan from 512 steps to 88 per core and widens every
instruction from 8 to 64 batch columns, amortizing the large fixed
per-instruction costs (ACT ~293ns, DVE ~150ns, PE ldweights ~104ns).

Contract: kernel(**inputs) takes FULL unsharded inputs, returns FULL
[B, T, TAGS, TAGS] crf_scores. Host only does weight re-layout, index
building, and output concatenation.
"""
import sys
import types
from contextlib import ExitStack

import ml_dtypes
import numpy as np

import concourse.bacc as bacc
import concourse.bass as bass
import concourse.mybir as mybir
import concourse.tile as tile
from concourse import library_config
from concourse.bass_utils import run_bass_kernel_spmd

# ---- problem dims (hardcoded per spec) ----
VOCAB = 30000
EMB = 256
HD = 128          # per-direction hidden
G4 = 512          # 4*HD gates
TAGS = 16
B, T = 64, 512
NCORES = 8

# ---- time-slice geometry ----
BC = B            # full batch on every core
OWN = 64          # owned timesteps per core
W = 16            # warmup steps (zero-state spin-up; h err ~3e-5 << budget)
SEQ = W + OWN     # scan steps per direction = 88
UB = SEQ // 8 + W // 8  # union blocks of 8 timesteps = 14
NBLK = SEQ // 8   # projection blocks per direction = 11
BOFF = W // 8     # b-window offset into union blocks = 3
NT = UB * 8 * BC  # tokens per core = 7168

BF16 = mybir.dt.bfloat16
F32 = mybir.dt.float32
I16 = mybir.dt.int16
AF = mybir.ActivationFunctionType
ALU = mybir.AluOpType

# gate order in reference (jnp.split): i, f, g, o. Reorder to [f, i, o, g]
# so one sigmoid covers [f|i|o] and g rides along as sigma(2x)
# (tanh(x) = 2*sigma(2x) - 1, g-weights doubled on host).
_PERM = np.concatenate([
    np.arange(128, 256),   # f
    np.arange(0, 128),     # i
    np.arange(384, 512),   # o
    np.arange(256, 384),   # g
])


def _ensure_ntff_hook():
    """The RL image's antenv lacks axon_hooks; inject it so trace=True works."""
    if "antenv.axon_hooks" in sys.modules:
        return
    mod = types.ModuleType("antenv.axon_hooks")
    mod._hook = None
    mod.set_axon_ntff_profile_hook = lambda h: setattr(mod, "_hook", h)
    mod.get_axon_ntff_profile_hook = lambda: mod._hook
    sys.modules["antenv.axon_hooks"] = mod
    try:
        import antenv
        antenv.axon_hooks = mod
        from trn_agent_boot.trn_boot import _ntff_profile_via_ctypes
        mod.set_axon_ntff_profile_hook(
            _ntff_profile_via_ctypes("/opt/axon/libaxon_pjrt.so"))
    except Exception:
        pass


# h-history piece boundaries per direction (tiles split so the emission
# epilogue's tile-granular deps bind near each chunk's true readiness).
# f writes cols ascending (col j+1 at step j), emission reads cols 25..88.
# b writes cols descending (col 87-j at step j), emission reads cols 0..63.
F_PIECES = [0, W + 33, W + 47, W + 57, SEQ + 1]
B_PIECES = [0, 10, 20, 34, SEQ + 1]


def build(seq: int = SEQ, bc: int = BC):
    nc = bacc.Bacc("TRN2", target_bir_lowering=False, debug=False)

    # ---- DRAM I/O ----
    emb_d = nc.dram_tensor("emb", [VOCAB, EMB], BF16, kind="ExternalInput")
    idx_d = nc.dram_tensor("idx", [128, NT // 16], I16, kind="ExternalInput")
    wihT_d = {d: nc.dram_tensor(f"wihT_{d}", [EMB, G4], BF16, kind="ExternalInput")
              for d in "fb"}
    whhT_d = {d: nc.dram_tensor(f"whhT_{d}", [HD, G4], BF16, kind="ExternalInput")
              for d in "fb"}
    bias_d = {d: nc.dram_tensor(f"bias_{d}", [128, 4], F32, kind="ExternalInput")
              for d in "fb"}
    biasw_d = {d: nc.dram_tensor(f"biasw_{d}", [128, 4], F32, kind="ExternalInput")
               for d in "fb"}
    woutT_d = nc.dram_tensor("woutT", [2, HD, TAGS], BF16, kind="ExternalInput")
    trans_d = nc.dram_tensor("trans", [128, TAGS * TAGS], F32, kind="ExternalInput")
    ident_d = nc.dram_tensor("ident", [128, 128], BF16, kind="ExternalInput")
    crf_d = nc.dram_tensor("crf", [OWN * bc, TAGS * TAGS], F32, kind="ExternalOutput")

    with tile.TileContext(nc) as tc, ExitStack() as ctx:
        const = ctx.enter_context(tc.tile_pool(name="const", bufs=1))
        big = ctx.enter_context(tc.tile_pool(name="big", bufs=1))

        # ---- persistent SBUF ----
        idx_sb = const.tile([128, NT // 16], I16)
        wihT = {d: const.tile([128, 2, G4], BF16, tag=f"wihT{d}", name=f"wihT{d}") for d in "fb"}
        whhT = {d: const.tile([HD, G4], BF16, tag=f"whhT{d}", name=f"whhT{d}") for d in "fb"}
        bias = {d: const.tile([128, 4], F32, tag=f"bias{d}", name=f"bias{d}") for d in "fb"}
        biasw = {d: const.tile([128, 4], F32, tag=f"biasw{d}", name=f"biasw{d}") for d in "fb"}
        woutT = const.tile([HD, 2, TAGS], BF16)
        trans = const.tile([128, TAGS * TAGS], F32)
        ident = const.tile([128, 128], BF16)
        C2 = {d: const.tile([128, 2 * bc], F32, tag=f"C2{d}", name=f"C2{d}")
              for d in "fb"}

        xT = big.tile([128, UB, 2, 512], BF16, tag="xT")
        zin = {d: big.tile([128, seq, 4 * bc], BF16, tag=f"zin{d}", name=f"zin{d}")
               for d in "fb"}

        # h histories as piece-split tiles
        hp = {}
        for d, bounds in (("f", F_PIECES), ("b", B_PIECES)):
            hp[d] = [(bounds[i], bounds[i + 1],
                      big.tile([128, bounds[i + 1] - bounds[i], bc], BF16,
                               tag=f"h{d}{i}", name=f"h{d}{i}"))
                     for i in range(len(bounds) - 1)]

        def hcol(d, col):
            for lo, hi, t in hp[d]:
                if lo <= col < hi:
                    return t[:, col - lo, :]
            raise AssertionError((d, col))

        def hspan(d, c0, n):
            for lo, hi, t in hp[d]:
                if lo <= c0 and c0 + n <= hi:
                    return t[:, c0 - lo:c0 - lo + n, :]
            raise AssertionError((d, c0, n))

        # ---- load constants ----
        nc.sync.dma_start(idx_sb[:], idx_d[:])
        for d in "fb":
            nc.sync.dma_start(wihT[d][:], wihT_d[d].rearrange("(k p) g -> p k g", p=128))
            nc.sync.dma_start(whhT[d][:], whhT_d[d][:])
            nc.sync.dma_start(bias[d][:], bias_d[d][:])
            nc.sync.dma_start(biasw[d][:], biasw_d[d][:])
        nc.sync.dma_start(woutT[:], woutT_d.rearrange("c h t -> h c t"))
        nc.sync.dma_start(trans[:], trans_d[:])
        nc.sync.dma_start(ident[:], ident_d[:])
        for d in "fb":
            nc.vector.memset(C2[d][:], 0.0)
        nc.gpsimd.memset(hcol("f", 0), 0.0)
        nc.gpsimd.memset(hcol("b", seq), 0.0)

        # ---- embedding gathers, in two-ended consumption order ----
        # f consumes union blocks 0,1,2,... ; b consumes 13,12,...
        gorder = []
        for k in range((UB + 1) // 2):
            gorder.append(k)
            if UB - 1 - k != k:
                gorder.append(UB - 1 - k)
        nc.gpsimd.load_library(library_config.mlp)
        for u in gorder:
            nc.gpsimd.dma_gather(
                xT[:, u, :, :], emb_d[:, :], idx_sb[:, 32 * u:32 * (u + 1)],
                512, 512, EMB, transpose=True)

        # ---- pools for projection + scan + emission (all open together
        # so no pool-release serialization mid-stream) ----
        # PSUM is 8 banks; pool bufs are bank-granular. matmul start=True
        # clears the WHOLE bank, so each direction's z needs its own bank.
        zpsum = ctx.enter_context(tc.tile_pool(name="zpsum", bufs=2, space="PSUM"))
        spsum = ctx.enter_context(tc.tile_pool(name="spsum", bufs=4, space="PSUM"))
        epsum = ctx.enter_context(tc.tile_pool(name="epsum", bufs=2, space="PSUM"))
        sZ = ctx.enter_context(tc.tile_pool(name="sZ", bufs=4))
        sS = ctx.enter_context(tc.tile_pool(name="sS", bufs=4))
        sP = ctx.enter_context(tc.tile_pool(name="sP", bufs=4))
        sT = ctx.enter_context(tc.tile_pool(name="sT", bufs=4))
        ecrf = ctx.enter_context(tc.tile_pool(name="ecrf", bufs=4))

        def proj_chunk(d, p, c):
            """Input projection for direction d, block p (8 timesteps),
            gate chunk c: 2 matmuls + 1 biased copyback. f reads union
            block p; b reads union block p+BOFF. Warmup blocks use the
            warm bias variant (edge cores force-kill f/i gates -> exact
            zero state)."""
            u = p if d == "f" else p + BOFF
            warm = (p < W // 8) if d == "f" else (p >= NBLK - W // 8)
            bsel = biasw[d] if warm else bias[d]
            zp = zpsum.tile([128, 512], F32, tag="zp")
            nc.tensor.matmul(zp[:], wihT[d][:, 0, 128 * c:128 * (c + 1)],
                             xT[:, u, 0, :], start=True, stop=False)
            nc.tensor.matmul(zp[:], wihT[d][:, 1, 128 * c:128 * (c + 1)],
                             xT[:, u, 1, :], start=False, stop=True)
            nc.scalar.activation(
                zin[d][:, 8 * p:8 * (p + 1), bc * c:bc * (c + 1)], zp[:],
                AF.Identity, bias=bsel[:, c:c + 1])

        # remaining projection work, spread chunk-by-chunk through the scan
        # (keeps the PE stream dense so the HAM clock stays warm); block
        # order follows two-ended consumption.
        proj_tasks = []
        fseq = list(range(2, NBLK))
        bseq = list(range(NBLK - 3, -1, -1))
        for i in range(NBLK - 2):
            proj_tasks += [("f", fseq[i], c) for c in range(4)]
            proj_tasks += [("b", bseq[i], c) for c in range(4)]
        proj_tasks.reverse()  # pop() from the front of the schedule

        # prime the pipeline: first two blocks per direction
        for p in (0, 1):
            for c in range(4):
                proj_chunk("f", p, c)
        for p in (NBLK - 1, NBLK - 2):
            for c in range(4):
                proj_chunk("b", p, c)

        # ---- the recurrent scan (fwd + bwd interleaved) ----
        def new_z(j):
            """Fresh psum tiles for step j with zin injected (identity mm),
            emitted one step ahead so gate mms fire as soon as h lands."""
            zt = {}
            for d, col in (("f", j), ("b", seq - 1 - j)):
                zt[d] = spsum.tile([128, 4 * bc], F32, tag="z", name=f"z{d}")
                nc.tensor.matmul(zt[d][:], ident[:],
                                 zin[d][:, col, :], start=True, stop=False)
            return zt

        z = new_z(0)
        for j in range(seq):
            u = seq - 1 - j
            # feed projection chunks into the instruction stream
            if proj_tasks:
                proj_chunk(*proj_tasks.pop())
            else:
                # HAM filler: a large-N matmul with no consumer keeps the
                # PE array streaming so the clock gate stays at 8/8
                # (small-N scan matmuls alone read as idle and the PE
                # drops to 1.2 GHz).
                fl = zpsum.tile([128, 512], F32, tag="zp")
                nc.tensor.matmul(fl[:], ident[:], xT[:, 0, 0, :],
                                 start=True, stop=True)
            for d, rd_col in (("f", j), ("b", u + 1)):
                for c in range(4):
                    nc.tensor.matmul(
                        z[d][:, bc * c:bc * (c + 1)],
                        whhT[d][:, 128 * c:128 * (c + 1)],
                        hcol(d, rd_col),
                        start=False, stop=(c == 3))
            z_cur, z = z, (new_z(j + 1) if j + 1 < seq else None)
            for d, wr_col in (("f", j + 1), ("b", u)):
                S = sS.tile([128, 4 * bc], F32, tag="S")
                nc.scalar.activation(S[:], z_cur[d][:], AF.Sigmoid)
                # gtilde = 2*sigma(2 z_g) - 1
                nc.vector.tensor_scalar(
                    C2[d][:, bc:2 * bc], S[:, 3 * bc:4 * bc], 2.0, -1.0,
                    ALU.mult, ALU.add)
                # [sf*c | si*gtilde]
                P2 = sP.tile([128, 2 * bc], F32, tag="P2")
                nc.vector.tensor_tensor(P2[:], S[:, 0:2 * bc], C2[d][:],
                                        ALU.mult)
                nc.vector.tensor_tensor(C2[d][:, 0:bc], P2[:, 0:bc],
                                        P2[:, bc:2 * bc], ALU.add)
                TC = sT.tile([128, bc], F32, tag="TC")
                nc.scalar.activation(TC[:], C2[d][:, 0:bc], AF.Tanh)
                nc.vector.tensor_tensor(hcol(d, wr_col),
                                        S[:, 2 * bc:3 * bc], TC[:], ALU.mult)

        # ---- emission + CRF broadcast-add + store ----
        # chunk m covers owned timesteps {2m, 2m+1}; ready at scan step
        # max(25+2m, 87-2m) -> emit middle-out.
        order = sorted(range(OWN // 2),
                       key=lambda m: max(W + 1 + 2 * m, seq - 1 - 2 * m))
        for m in order:
            e = epsum.tile([128, TAGS], F32, tag="e")
            nc.tensor.matmul(e[:], hspan("f", W + 1 + 2 * m, 2),
                             woutT[:, 0, :], start=True, stop=False)
            nc.tensor.matmul(e[:], hspan("b", 2 * m, 2),
                             woutT[:, 1, :], start=False, stop=True)
            crf_sb = ecrf.tile([128, TAGS * TAGS], F32, tag="crf")
            e_b = e[:, None, :].to_broadcast([128, TAGS, TAGS])
            nc.vector.tensor_tensor(crf_sb[:], e_b, trans[:], ALU.add)
            nc.sync.dma_start(crf_d[128 * m:128 * (m + 1), :], crf_sb[:])

    nc.compile()
    _assert_ldw_pairing(nc)
    return nc


def _assert_ldw_pairing(nc):
    """Every non-self-loading matmul must directly follow an InstLdweights
    whose weights AP matches the matmul's weights operand."""
    for f in nc.m.functions:
        for bb in f.blocks:
            prev_pe = None
            for ins in bb.instructions:
                if ins.engine != mybir.EngineType.PE:
                    continue
                if isinstance(ins, mybir.InstMatmult) and ins.ldweights is False:
                    assert isinstance(prev_pe, mybir.InstLdweights), (
                        f"{ins.name}: non-self-loading matmul not preceded by "
                        f"ldweights (got {type(prev_pe).__name__})")
                    assert repr(prev_pe.ins[0]) == repr(ins.ins[1]), (
                        f"{ins.name}: weights mismatch with {prev_pe.name}")
                prev_pe = ins


_CACHE = {}


def _get_nc():
    if "nc" not in _CACHE:
        _CACHE["nc"] = build()
    return _CACHE["nc"]


def _prep_dir(w_ih, w_hh, b):
    w_ih = np.asarray(w_ih, np.float32)[_PERM].copy()
    w_hh = np.asarray(w_hh, np.float32)[_PERM].copy()
    b = np.asarray(b, np.float32)[_PERM].copy()
    w_ih[384:512] *= 2.0
    w_hh[384:512] *= 2.0
    b[384:512] *= 2.0
    wihT = np.ascontiguousarray(w_ih.T).astype(ml_dtypes.bfloat16)
    whhT = np.ascontiguousarray(w_hh.T).astype(ml_dtypes.bfloat16)
    bias = np.ascontiguousarray(b.reshape(4, 128).T).astype(np.float32)
    return wihT, whhT, bias


def make_in_maps(sentences, embedding, W_ih_f, W_hh_f, b_f, W_ih_b, W_hh_b,
                 b_b, W_out, b_out, transition):
    emb = np.asarray(embedding, np.float32).astype(ml_dtypes.bfloat16)
    wihT_f, whhT_f, bias_f = _prep_dir(W_ih_f, W_hh_f, b_f)
    wihT_b, whhT_b, bias_b = _prep_dir(W_ih_b, W_hh_b, b_b)
    wo = np.asarray(W_out, np.float32)  # [16, 256]
    woutT = np.stack([np.ascontiguousarray(wo[:, :128].T),
                      np.ascontiguousarray(wo[:, 128:].T)])
    woutT = woutT.astype(ml_dtypes.bfloat16)  # [2, 128, 16]
    trans_aug = (np.asarray(transition, np.float32)
                 + np.asarray(b_out, np.float32)[None, :]).reshape(-1)
    trans_rep = np.ascontiguousarray(
        np.broadcast_to(trans_aug, (128, 256))).astype(np.float32)
    ident = np.eye(128, dtype=ml_dtypes.bfloat16)

    sent = np.asarray(sentences).astype(np.int64)
    in_maps = []
    for s in range(NCORES):
        # union token window [64s - W, 64s + SEQ), clamped; (t, b) order
        ts = np.clip(np.arange(64 * s - W, 64 * s + SEQ), 0, T - 1)
        toks = sent[:, ts].T.reshape(-1)  # [NT] t-major
        idx = np.tile(toks.reshape(NT // 16, 16).T.astype(np.int16), (8, 1))
        # warm bias kills f/i gates only where the warmup window falls
        # outside the real sequence (slice edges)
        bwf = bias_f.copy()
        bwb = bias_b.copy()
        if s == 0:
            bwf[:, 0:2] = -30000.0
        if s == NCORES - 1:
            bwb[:, 0:2] = -30000.0
        in_maps.append({
            "emb": emb, "idx": idx,
            "wihT_f": wihT_f, "wihT_b": wihT_b,
            "whhT_f": whhT_f, "whhT_b": whhT_b,
            "bias_f": bias_f, "bias_b": bias_b,
            "biasw_f": bwf, "biasw_b": bwb,
            "woutT": woutT, "trans": trans_rep, "ident": ident,
        })
    return in_maps


def assemble_out(results):
    out = np.empty((B, T, TAGS, TAGS), np.float32)
    for s in range(NCORES):
        crf = results[s]["crf"].reshape(OWN, B, TAGS, TAGS)
        out[:, OWN * s:OWN * (s + 1)] = crf.transpose(1, 0, 2, 3)
    return out


def kernel(**inputs):
    _ensure_ntff_hook()
    nc = _get_nc()
    in_maps = make_in_maps(**inputs)
    res = run_bass_kernel_spmd(nc, in_maps, list(range(NCORES)))
    return assemble_out(res.results)
